# revision 25
# baseline (speedup 1.0000x reference)
"""Trainium2 Bass kernel for nn_CombinedBandPassFilter.

Computes y[b, 0, f, t] = sum_k x[b, 0, t+k-384] * kernels[f, k]  (conv1d,
'same' padding, K=769, 40 filters, B=32, T=32768).

Strategy (8 NeuronCores, batch-sharded: 4 batches x all 40 filters per core):
  Block-Toeplitz matmul formulation. Output chunked t = 256*a + s with
  a in [0,128) as the PSUM partition dim and s in [0,256) as the free dim:

      y[256a + s] = sum_j sum_r x[256a + 128j + OFF + r] * W_j[r, s]
      W_j[r, s]   = h[384 + 128j + OFF + r - s]

  The stationary matmul operand is a [128, 128] stride-2-column slice of x
  stored block-column-major in SBUF (Xmat[r, c] = x[128c + SHIFT + r]); the
  moving operand is the precomputed filter-Toeplitz block W_j. PSUM
  accumulates over j. Per-filter offset OFF in {0, -64} (two x layouts)
  minimizes the j-block count to ceil((2m + 256)/128) for true tap
  half-width m, exploiting the wildly varying filter supports (19..769).

  Filters with identical (OFF, jmin, J) signatures are paired side by side
  into N=512 matmul streams sharing one stationary load: the 30 short
  filters form 15 pairs, the 6 mid filters 3 pairs, l=4/l=5 one pair; only
  the two longest filters run solo at N=256.  Every core executes the
  identical schedule on its own 4 batches, so the SPMD program has zero
  per-core padding.  DMA traffic is spread across the SP/ACT HWDGE queues
  and the gpsimd SWDGE queue; PSUM evacuation is split between the vector
  and scalar engines.
"""

import math
import os
import numpy as np

B = 32
T = 32768
KLEN = 769
PADK = 384
N = 256          # output chunk size = psum free dim
NCORE = 8
NF = 40
GB = B // NCORE  # batches per core

_COMPILED = {}
LAST_RESULT = None   # BassKernelResults of the most recent run (for test.py)


def _dtype_cfg():
    import concourse.mybir as mybir
    kind = os.environ.get("KERNEL_DTYPE", "f16")
    if kind == "f32r":
        return kind, mybir.dt.float32r, np.float32
    if kind == "f16":
        return kind, mybir.dt.float16, np.float16
    if kind == "bf16":
        import ml_dtypes
        return kind, mybir.dt.bfloat16, ml_dtypes.bfloat16
    raise ValueError(kind)


# ---------------------------------------------------------------- filters ---
def _design_filter(fs, low_hz, high_hz, cycle):
    n_taps = int(cycle * fs / low_hz)
    if n_taps % 2 == 0:
        n_taps += 1
    m = (n_taps - 1) / 2.0
    k = np.arange(n_taps) - m
    fl, fh = low_hz / fs, high_hz / fs
    h = 2.0 * fh * np.sinc(2.0 * fh * k) - 2.0 * fl * np.sinc(2.0 * fl * k)
    w = 0.54 - 0.46 * np.cos(2.0 * np.pi * np.arange(n_taps) / (n_taps - 1))
    h = h * w
    fc = 0.5 * (low_hz + high_hz) / fs
    resp = np.abs(np.sum(h * np.exp(-2j * np.pi * fc * k)))
    return (h / resp).astype(np.float32)


def _build_kernels():
    FS, CYCLE_PHA, CYCLE_AMP = 512, 3, 6
    pha = [(l, l + 2) for l in range(2, 22)]
    amp = [(l, l + 20) for l in range(60, 160, 5)]
    filters = [_design_filter(FS, l, h, CYCLE_PHA) for (l, h) in pha]
    filters += [_design_filter(FS, l, h, CYCLE_AMP) for (l, h) in amp]
    max_len = max(f.shape[0] for f in filters)
    padded = []
    for f in filters:
        pad = max_len - f.shape[0]
        padded.append(np.pad(f, (pad // 2, pad - pad // 2)))
    return np.stack(padded).astype(np.float32)


# ------------------------------------------------------------------- plan ---
class Plan:
    pass


def _make_plan(kernels):
    """Per-filter coverage plan + pairing schedule (same for every core)."""
    per_f = []
    for f in range(NF):
        nz = np.nonzero(kernels[f])[0]
        m = int(max(PADK - nz[0], nz[-1] - PADK)) if len(nz) else 0
        best = None
        for fam, OFF in ((0, 0), (1, -64)):
            jmin = math.floor((-m - OFF) / 128)
            jmax = math.floor((255 + m - OFF) / 128)
            nj = jmax - jmin + 1
            if best is None or nj < best[4]:
                best = (fam, OFF, jmin, jmax, nj)
        per_f.append(best)

    # group filters by signature (fam, jmin, J); pair within each class
    from collections import defaultdict
    classes = defaultdict(list)
    for f in range(NF):
        fam, OFF, jmin, jmax, nj = per_f[f]
        classes[(fam, jmin, nj)].append(f)

    schedule = []   # (filters_tuple, fam, jmin, J, width)
    for (fam, jmin, nj), fs in sorted(classes.items(), key=lambda kv: kv[0][2]):
        i = 0
        while i + 1 < len(fs):
            schedule.append(((fs[i], fs[i + 1]), fam, jmin, nj, 2 * N))
            i += 2
        if i < len(fs):
            schedule.append(((fs[i],), fam, jmin, nj, N))

    p = Plan()
    p.per_f = per_f
    p.schedule = schedule
    # W layout: schedule-order, per step a [128, width] block
    p.w_steps = []   # per schedule idx: list of col offsets per step
    col = 0
    for (fls, fam, jmin, J, width) in schedule:
        cols = []
        for t in range(J):
            cols.append(col)
            col += width
        p.w_steps.append(cols)
    p.w_cols = col

    # x layout ranges per family
    p.fam_off = [0, -64]
    p.fam_cmin = []
    p.fam_C = []
    for fam in range(2):
        ents = [s for s in schedule if s[1] == fam]
        jmin = min(s[2] for s in ents)
        jmax = max(s[2] + s[3] - 1 for s in ents)
        p.fam_cmin.append(jmin)
        p.fam_C.append(254 + jmax - jmin + 1)
    p.xb_cols = p.fam_C[0] + p.fam_C[1]
    return p


# -------------------------------------------------------------- host prep ---
def _prep_x(x, plan, npdt):
    """Block-column-major x, per-batch [famA | famB]: [128, B*xb_cols]."""
    xf = np.ascontiguousarray(x.reshape(B, T), dtype=np.float32)
    LPAD = 1024
    xp = np.zeros((B, LPAD + T + LPAD), np.float32)
    xp[:, LPAD:LPAD + T] = xf
    fams = []
    for fam in range(2):
        C = plan.fam_C[fam]
        start = LPAD + 128 * plan.fam_cmin[fam] + plan.fam_off[fam]
        v = np.lib.stride_tricks.as_strided(
            xp[:, start:], shape=(B, C, 128),
            strides=(xp.strides[0], 512, 4))
        fams.append(v)
    out = np.concatenate(fams, axis=1)            # [B, xb_cols, 128]
    out = np.ascontiguousarray(out.transpose(2, 0, 1), dtype=npdt)
    return out.reshape(128, B * plan.xb_cols)


def _build_W(h, OFF, j):
    r = np.arange(128)[:, None]
    s = np.arange(N)[None, :]
    k = PADK + 128 * j + OFF + r - s
    valid = (k >= 0) & (k < KLEN)
    W = np.zeros((128, N), np.float32)
    W[valid] = h[np.clip(k, 0, KLEN - 1)][valid]
    return W


def _prep_w(kernels, plan, npdt):
    """Moving-operand blocks, identical for all cores: [128, w_cols]."""
    Wc = np.zeros((128, plan.w_cols), np.float32)
    for gi, (fls, fam, jmin, J, width) in enumerate(plan.schedule):
        OFF = plan.fam_off[fam]
        for t in range(J):
            j = jmin + t
            col = plan.w_steps[gi][t]
            for si, f in enumerate(fls):
                if plan.per_f[f][2] <= j <= plan.per_f[f][3]:
                    Wc[:, col + si * N:col + (si + 1) * N] = \
                        _build_W(kernels[f], OFF, j)
    return np.ascontiguousarray(Wc.astype(npdt))


# ---------------------------------------------------------------- program ---
def _build_program(plan, mmdt):
    import concourse.bacc as bacc
    import concourse.mybir as mybir
    from concourse.tile import TileContext

    F32 = mybir.dt.float32

    nc = bacc.Bacc("TRN2", target_bir_lowering=False)
    outdt = F32 if os.environ.get("KERNEL_OUT", "f16") == "f32" else mybir.dt.float16
    x_d = nc.dram_tensor("x", [128, GB * plan.xb_cols], mmdt,
                         kind="ExternalInput")
    w_d = nc.dram_tensor("w", [128, plan.w_cols], mmdt, kind="ExternalInput")
    y_d = nc.dram_tensor("y", [GB, NF, T], outdt, kind="ExternalOutput")
    y_ap = y_d.ap()

    rr = [0]

    with TileContext(nc) as tc:
        engs3 = (nc.sync, nc.scalar, nc.gpsimd)
        with (
            tc.tile_pool(name="wconst", bufs=1) as wpool,
            tc.tile_pool(name="xconst", bufs=1) as xpool,
            tc.tile_pool(name="psum", bufs=8, space="PSUM") as ppool,
            tc.tile_pool(name="ev", bufs=10) as epool,
        ):
            # One W tile per schedule group (separate tiles -> matmuls only
            # wait for their own group's DMA), round-robin on the 3 queues.
            # W streams exclusively on sync+gpsimd (clean FIFO, no
            # head-of-line blocking from the small output descriptors);
            # early outputs go to scalar only, late ones round-robin.
            x_s = xpool.tile([128, GB * plan.xb_cols], mmdt)
            nc.sync.dma_start(x_s[:], x_d[:])
            w_tiles = []
            for gi, (fls, fam, jmin, J, width) in enumerate(plan.schedule):
                lo = plan.w_steps[gi][0]
                hi = plan.w_steps[gi][-1] + width
                wt = wpool.tile([128, hi - lo], mmdt, tag=f"w{gi}")
                (nc.sync if gi % 2 == 0 else nc.gpsimd).dma_start(
                    wt[:], w_d[:, lo:hi])
                w_tiles.append(wt)
            ngrp = len(plan.schedule)

            # groups outer, batches inner: each W tile feeds 4 batches of
            # matmuls as soon as it lands, so W streaming stays ahead of PE.
            for gi, (fls, fam, jmin, J, width) in enumerate(plan.schedule):
                cmin = plan.fam_cmin[fam]
                pss = [ppool.tile([128, width], F32, tag="ps",
                                  name=f"ps_{gi}_{bb}") for bb in range(GB)]
                for t in range(J):
                    j = jmin + t
                    rhs = w_tiles[gi][:, t * width:(t + 1) * width]
                    for b in range(GB):
                        fbase = (b * plan.xb_cols
                                 + (plan.fam_C[0] if fam == 1 else 0))
                        col0 = fbase + (j - cmin)
                        lhsT = x_s[:, col0:col0 + 255:2]
                        nc.tensor.matmul(pss[b][:], lhsT, rhs,
                                         start=(t == 0), stop=(t == J - 1))
                for b in range(GB):
                    ev = epool.tile([128, width], outdt, tag="ev")
                    if (gi * GB + b) % 5 != 4:
                        nc.vector.tensor_copy(ev[:], pss[b][:])
                    else:
                        nc.scalar.copy(ev[:], pss[b][:])
                    for si, f in enumerate(fls):
                        yv = y_ap[b, f].rearrange("(a n) -> a n", n=N)
                        if gi < ngrp * 3 // 5:
                            eng = nc.scalar
                        else:
                            eng = engs3[rr[0] % 3]
                            rr[0] += 1
                        eng.dma_start(yv, ev[:, si * N:(si + 1) * N])
    nc.finalize()
    return nc


# ----------------------------------------------------------------- kernel ---
def kernel(x, kernels=None):
    global LAST_RESULT
    from concourse.bass_utils import run_bass_kernel_spmd

    x = np.asarray(x, dtype=np.float32)
    if kernels is None:
        kernels = _build_kernels()
    kernels = np.asarray(kernels, dtype=np.float32)
    assert x.shape == (B, 1, T) and kernels.shape == (NF, KLEN)

    kind, mmdt, npdt = _dtype_cfg()
    ckey = (kind, os.environ.get("KERNEL_OUT", "f16"))
    if ckey not in _COMPILED:
        plan = _make_plan(kernels)
        nc = _build_program(plan, mmdt)
        _COMPILED[ckey] = (nc, plan)
    nc, plan = _COMPILED[ckey]

    xh = _prep_x(x, plan, npdt)
    w = _prep_w(kernels, plan, npdt)
    gcols = GB * plan.xb_cols
    in_maps = [{"x": np.ascontiguousarray(xh[:, c * gcols:(c + 1) * gcols]),
                "w": w} for c in range(NCORE)]

    trace = bool(int(os.environ.get("KERNEL_TRACE", "0")))
    try:
        res = run_bass_kernel_spmd(nc, in_maps, core_ids=list(range(NCORE)),
                                   trace=trace)
    except Exception:
        if not trace:
            raise
        res = run_bass_kernel_spmd(nc, in_maps, core_ids=list(range(NCORE)),
                                   trace=False)
    LAST_RESULT = res

    out = np.empty((B, 1, NF, T), np.float32)
    for c in range(NCORE):
        out[c * GB:(c + 1) * GB, 0] = res.results[c]["y"].astype(np.float32)
    return out


# revision 26
# speedup vs baseline: 1.1844x; 1.1844x over previous
"""Trainium2 Bass kernel for nn_CombinedBandPassFilter.

Computes y[b, 0, f, t] = sum_k x[b, 0, t+k-384] * kernels[f, k]  (conv1d,
'same' padding, K=769, 40 filters, B=32, T=32768).

Strategy (8 NeuronCores, batch-sharded: 4 batches x all 40 filters per core):
  Block-Toeplitz matmul formulation. Output chunked t = 256*a + s with
  a in [0,128) as the PSUM partition dim and s in [0,256) as the free dim:

      y[256a + s] = sum_j sum_r x[256a + 128j + OFF + r] * W_j[r, s]
      W_j[r, s]   = h[384 + 128j + OFF + r - s]

  The stationary matmul operand is a [128, 128] stride-2-column slice of x
  stored block-column-major in SBUF (Xmat[r, c] = x[128c + SHIFT + r]); the
  moving operand is the precomputed filter-Toeplitz block W_j. PSUM
  accumulates over j. Per-filter offset OFF in {0, -64} (two x layouts)
  minimizes the j-block count to ceil((2m + 256)/128) for true tap
  half-width m, exploiting the wildly varying filter supports (19..769).

  Filters with identical (OFF, jmin, J) signatures are paired side by side
  into N=512 matmul streams sharing one stationary load: the 30 short
  filters form 15 pairs, the 6 mid filters 3 pairs, l=4/l=5 one pair; only
  the two longest filters run solo at N=256.  Every core executes the
  identical schedule on its own 4 batches, so the SPMD program has zero
  per-core padding.  DMA traffic is spread across the SP/ACT HWDGE queues
  and the gpsimd SWDGE queue; PSUM evacuation is split between the vector
  and scalar engines.
"""

import math
import os
import numpy as np

B = 32
T = 32768
KLEN = 769
PADK = 384
N = 256          # output chunk size = psum free dim
NCORE = 8
NF = 40
GB = B // NCORE  # batches per core

_COMPILED = {}
LAST_RESULT = None   # BassKernelResults of the most recent run (for test.py)


def _dtype_cfg():
    import concourse.mybir as mybir
    kind = os.environ.get("KERNEL_DTYPE", "f16")
    if kind == "f32r":
        return kind, mybir.dt.float32r, np.float32
    if kind == "f16":
        return kind, mybir.dt.float16, np.float16
    if kind == "bf16":
        import ml_dtypes
        return kind, mybir.dt.bfloat16, ml_dtypes.bfloat16
    raise ValueError(kind)


# ---------------------------------------------------------------- filters ---
def _design_filter(fs, low_hz, high_hz, cycle):
    n_taps = int(cycle * fs / low_hz)
    if n_taps % 2 == 0:
        n_taps += 1
    m = (n_taps - 1) / 2.0
    k = np.arange(n_taps) - m
    fl, fh = low_hz / fs, high_hz / fs
    h = 2.0 * fh * np.sinc(2.0 * fh * k) - 2.0 * fl * np.sinc(2.0 * fl * k)
    w = 0.54 - 0.46 * np.cos(2.0 * np.pi * np.arange(n_taps) / (n_taps - 1))
    h = h * w
    fc = 0.5 * (low_hz + high_hz) / fs
    resp = np.abs(np.sum(h * np.exp(-2j * np.pi * fc * k)))
    return (h / resp).astype(np.float32)


def _build_kernels():
    FS, CYCLE_PHA, CYCLE_AMP = 512, 3, 6
    pha = [(l, l + 2) for l in range(2, 22)]
    amp = [(l, l + 20) for l in range(60, 160, 5)]
    filters = [_design_filter(FS, l, h, CYCLE_PHA) for (l, h) in pha]
    filters += [_design_filter(FS, l, h, CYCLE_AMP) for (l, h) in amp]
    max_len = max(f.shape[0] for f in filters)
    padded = []
    for f in filters:
        pad = max_len - f.shape[0]
        padded.append(np.pad(f, (pad // 2, pad - pad // 2)))
    return np.stack(padded).astype(np.float32)


# ------------------------------------------------------------------- plan ---
class Plan:
    pass


def _make_plan(kernels):
    """Per-filter coverage plan + pairing schedule (same for every core)."""
    per_f = []
    for f in range(NF):
        nz = np.nonzero(kernels[f])[0]
        m = int(max(PADK - nz[0], nz[-1] - PADK)) if len(nz) else 0
        best = None
        for fam, OFF in ((0, 0), (1, -64)):
            jmin = math.floor((-m - OFF) / 128)
            jmax = math.floor((255 + m - OFF) / 128)
            nj = jmax - jmin + 1
            if best is None or nj < best[4]:
                best = (fam, OFF, jmin, jmax, nj)
        per_f.append(best)

    # group filters by signature (fam, jmin, J); pair within each class
    from collections import defaultdict
    classes = defaultdict(list)
    for f in range(NF):
        fam, OFF, jmin, jmax, nj = per_f[f]
        classes[(fam, jmin, nj)].append(f)

    schedule = []   # (filters_tuple, fam, jmin, J, width)
    for (fam, jmin, nj), fs in sorted(classes.items(), key=lambda kv: kv[0][2]):
        i = 0
        while i + 1 < len(fs):
            schedule.append(((fs[i], fs[i + 1]), fam, jmin, nj, 2 * N))
            i += 2
        if i < len(fs):
            schedule.append(((fs[i],), fam, jmin, nj, N))

    p = Plan()
    p.per_f = per_f
    p.schedule = schedule
    # W layout: schedule-order, per step a [128, width] block
    p.w_steps = []   # per schedule idx: list of col offsets per step
    col = 0
    for (fls, fam, jmin, J, width) in schedule:
        cols = []
        for t in range(J):
            cols.append(col)
            col += width
        p.w_steps.append(cols)
    p.w_cols = col

    # x layout ranges per family
    p.fam_off = [0, -64]
    p.fam_cmin = []
    p.fam_C = []
    for fam in range(2):
        ents = [s for s in schedule if s[1] == fam]
        jmin = min(s[2] for s in ents)
        jmax = max(s[2] + s[3] - 1 for s in ents)
        p.fam_cmin.append(jmin)
        p.fam_C.append(254 + jmax - jmin + 1)
    p.xb_cols = p.fam_C[0] + p.fam_C[1]
    return p


# -------------------------------------------------------------- host prep ---
def _prep_x(x, plan, npdt):
    """Block-column-major x, per-batch [famA | famB]: [128, B*xb_cols]."""
    xf = np.ascontiguousarray(x.reshape(B, T), dtype=np.float32)
    LPAD = 1024
    xp = np.zeros((B, LPAD + T + LPAD), np.float32)
    xp[:, LPAD:LPAD + T] = xf
    fams = []
    for fam in range(2):
        C = plan.fam_C[fam]
        start = LPAD + 128 * plan.fam_cmin[fam] + plan.fam_off[fam]
        v = np.lib.stride_tricks.as_strided(
            xp[:, start:], shape=(B, C, 128),
            strides=(xp.strides[0], 512, 4))
        fams.append(v)
    out = np.concatenate(fams, axis=1)            # [B, xb_cols, 128]
    out = np.ascontiguousarray(out.transpose(2, 0, 1), dtype=npdt)
    return out.reshape(128, B * plan.xb_cols)


def _build_W(h, OFF, j):
    r = np.arange(128)[:, None]
    s = np.arange(N)[None, :]
    k = PADK + 128 * j + OFF + r - s
    valid = (k >= 0) & (k < KLEN)
    W = np.zeros((128, N), np.float32)
    W[valid] = h[np.clip(k, 0, KLEN - 1)][valid]
    return W


def _prep_w(kernels, plan, npdt):
    """Moving-operand blocks, identical for all cores: [128, w_cols]."""
    Wc = np.zeros((128, plan.w_cols), np.float32)
    for gi, (fls, fam, jmin, J, width) in enumerate(plan.schedule):
        OFF = plan.fam_off[fam]
        for t in range(J):
            j = jmin + t
            col = plan.w_steps[gi][t]
            for si, f in enumerate(fls):
                if plan.per_f[f][2] <= j <= plan.per_f[f][3]:
                    Wc[:, col + si * N:col + (si + 1) * N] = \
                        _build_W(kernels[f], OFF, j)
    return np.ascontiguousarray(Wc.astype(npdt))


# ---------------------------------------------------------------- program ---
def _build_program(plan, mmdt):
    import concourse.bacc as bacc
    import concourse.mybir as mybir
    from concourse.tile import TileContext

    F32 = mybir.dt.float32

    nc = bacc.Bacc("TRN2", target_bir_lowering=False)
    outdt = F32 if os.environ.get("KERNEL_OUT", "f16") == "f32" else mybir.dt.float16
    x_d = nc.dram_tensor("x", [128, GB * plan.xb_cols], mmdt,
                         kind="ExternalInput")
    w_d = nc.dram_tensor("w", [128, plan.w_cols], mmdt, kind="ExternalInput")
    y_d = nc.dram_tensor("y", [GB, NF, T], outdt, kind="ExternalOutput")
    y_ap = y_d.ap()

    rr = [0]

    with TileContext(nc) as tc:
        engs3 = (nc.sync, nc.scalar, nc.gpsimd)
        with (
            tc.tile_pool(name="wconst", bufs=1) as wpool,
            tc.tile_pool(name="xconst", bufs=1) as xpool,
            tc.tile_pool(name="psum", bufs=8, space="PSUM") as ppool,
            tc.tile_pool(name="ev", bufs=10) as epool,
        ):
            # One W tile per schedule group (separate tiles -> matmuls only
            # wait for their own group's DMA), round-robin on the 3 queues.
            # W streams exclusively on sync+gpsimd (clean FIFO, no
            # head-of-line blocking from the small output descriptors);
            # early outputs go to scalar only, late ones round-robin.
            x_s = xpool.tile([128, GB * plan.xb_cols], mmdt)
            nc.sync.dma_start(x_s[:], x_d[:])
            w_tiles = []
            for gi, (fls, fam, jmin, J, width) in enumerate(plan.schedule):
                lo = plan.w_steps[gi][0]
                hi = plan.w_steps[gi][-1] + width
                wt = wpool.tile([128, hi - lo], mmdt, tag=f"w{gi}")
                (nc.sync if gi % 2 == 0 else nc.gpsimd).dma_start(
                    wt[:], w_d[:, lo:hi])
                w_tiles.append(wt)
            ngrp = len(plan.schedule)

            # groups outer, batches inner: each W tile feeds 4 batches of
            # matmuls as soon as it lands, so W streaming stays ahead of PE.
            for gi, (fls, fam, jmin, J, width) in enumerate(plan.schedule):
                cmin = plan.fam_cmin[fam]
                pss = [ppool.tile([128, width], F32, tag="ps",
                                  name=f"ps_{gi}_{bb}") for bb in range(GB)]
                for t in range(J):
                    j = jmin + t
                    rhs = w_tiles[gi][:, t * width:(t + 1) * width]
                    for b in range(GB):
                        fbase = (b * plan.xb_cols
                                 + (plan.fam_C[0] if fam == 1 else 0))
                        col0 = fbase + (j - cmin)
                        lhsT = x_s[:, col0:col0 + 255:2]
                        nc.tensor.matmul(pss[b][:], lhsT, rhs,
                                         start=(t == 0), stop=(t == J - 1))
                for b in range(GB):
                    ev = epool.tile([128, width], outdt, tag="ev")
                    if (gi * GB + b) % 5 != 4:
                        nc.vector.tensor_copy(ev[:], pss[b][:])
                    else:
                        nc.scalar.copy(ev[:], pss[b][:])
                    for si, f in enumerate(fls):
                        yv = y_ap[b, f].rearrange("(a n) -> a n", n=N)
                        eng = engs3[rr[0] % 3]
                        rr[0] += 1
                        eng.dma_start(yv, ev[:, si * N:(si + 1) * N])
    nc.finalize()
    return nc


# ----------------------------------------------------------------- kernel ---
def kernel(x, kernels=None):
    global LAST_RESULT
    from concourse.bass_utils import run_bass_kernel_spmd

    x = np.asarray(x, dtype=np.float32)
    if kernels is None:
        kernels = _build_kernels()
    kernels = np.asarray(kernels, dtype=np.float32)
    assert x.shape == (B, 1, T) and kernels.shape == (NF, KLEN)

    kind, mmdt, npdt = _dtype_cfg()
    ckey = (kind, os.environ.get("KERNEL_OUT", "f16"))
    if ckey not in _COMPILED:
        plan = _make_plan(kernels)
        nc = _build_program(plan, mmdt)
        _COMPILED[ckey] = (nc, plan)
    nc, plan = _COMPILED[ckey]

    xh = _prep_x(x, plan, npdt)
    w = _prep_w(kernels, plan, npdt)
    gcols = GB * plan.xb_cols
    in_maps = [{"x": np.ascontiguousarray(xh[:, c * gcols:(c + 1) * gcols]),
                "w": w} for c in range(NCORE)]

    trace = bool(int(os.environ.get("KERNEL_TRACE", "0")))
    try:
        res = run_bass_kernel_spmd(nc, in_maps, core_ids=list(range(NCORE)),
                                   trace=trace)
    except Exception:
        if not trace:
            raise
        res = run_bass_kernel_spmd(nc, in_maps, core_ids=list(range(NCORE)),
                                   trace=False)
    LAST_RESULT = res

    out = np.empty((B, 1, NF, T), np.float32)
    for c in range(NCORE):
        out[c * GB:(c + 1) * GB, 0] = res.results[c]["y"].astype(np.float32)
    return out


# revision 28
# speedup vs baseline: 1.3609x; 1.1491x over previous
"""Trainium2 Bass kernel for nn_CombinedBandPassFilter.

Computes y[b, 0, f, t] = sum_k x[b, 0, t+k-384] * kernels[f, k]  (conv1d,
'same' padding, K=769, 40 filters, B=32, T=32768).

Strategy (8 NeuronCores, batch-sharded: 4 batches x all 40 filters per core):
  Block-Toeplitz matmul formulation. Output chunked t = 256*a + s with
  a in [0,128) as the PSUM partition dim and s in [0,256) as the free dim:

      y[256a + s] = sum_j sum_r x[256a + 128j + OFF + r] * W_j[r, s]
      W_j[r, s]   = h[384 + 128j + OFF + r - s]

  The stationary matmul operand is a [128, 128] stride-2-column slice of x
  stored block-column-major in SBUF (Xmat[r, c] = x[128c + SHIFT + r]); the
  moving operand is the precomputed filter-Toeplitz block W_j. PSUM
  accumulates over j. Per-filter offset OFF in {0, -64} (two x layouts)
  minimizes the j-block count to ceil((2m + 256)/128) for true tap
  half-width m, exploiting the wildly varying filter supports (19..769).

  Filters with identical (OFF, jmin, J) signatures are paired side by side
  into N=512 matmul streams sharing one stationary load: the 30 short
  filters form 15 pairs, the 6 mid filters 3 pairs, l=4/l=5 one pair; only
  the two longest filters run solo at N=256.  Every core executes the
  identical schedule on its own 4 batches, so the SPMD program has zero
  per-core padding.  DMA traffic is spread across the SP/ACT HWDGE queues
  and the gpsimd SWDGE queue; PSUM evacuation is split between the vector
  and scalar engines.
"""

import math
import os
import numpy as np

B = 32
T = 32768
KLEN = 769
PADK = 384
N = 256          # output chunk size = psum free dim
NCORE = 8
NF = 40
GB = B // NCORE  # batches per core

_COMPILED = {}
LAST_RESULT = None   # BassKernelResults of the most recent run (for test.py)


def _dtype_cfg():
    import concourse.mybir as mybir
    kind = os.environ.get("KERNEL_DTYPE", "f16")
    if kind == "f32r":
        return kind, mybir.dt.float32r, np.float32
    if kind == "f16":
        return kind, mybir.dt.float16, np.float16
    if kind == "bf16":
        import ml_dtypes
        return kind, mybir.dt.bfloat16, ml_dtypes.bfloat16
    raise ValueError(kind)


# ---------------------------------------------------------------- filters ---
def _design_filter(fs, low_hz, high_hz, cycle):
    n_taps = int(cycle * fs / low_hz)
    if n_taps % 2 == 0:
        n_taps += 1
    m = (n_taps - 1) / 2.0
    k = np.arange(n_taps) - m
    fl, fh = low_hz / fs, high_hz / fs
    h = 2.0 * fh * np.sinc(2.0 * fh * k) - 2.0 * fl * np.sinc(2.0 * fl * k)
    w = 0.54 - 0.46 * np.cos(2.0 * np.pi * np.arange(n_taps) / (n_taps - 1))
    h = h * w
    fc = 0.5 * (low_hz + high_hz) / fs
    resp = np.abs(np.sum(h * np.exp(-2j * np.pi * fc * k)))
    return (h / resp).astype(np.float32)


def _build_kernels():
    FS, CYCLE_PHA, CYCLE_AMP = 512, 3, 6
    pha = [(l, l + 2) for l in range(2, 22)]
    amp = [(l, l + 20) for l in range(60, 160, 5)]
    filters = [_design_filter(FS, l, h, CYCLE_PHA) for (l, h) in pha]
    filters += [_design_filter(FS, l, h, CYCLE_AMP) for (l, h) in amp]
    max_len = max(f.shape[0] for f in filters)
    padded = []
    for f in filters:
        pad = max_len - f.shape[0]
        padded.append(np.pad(f, (pad // 2, pad - pad // 2)))
    return np.stack(padded).astype(np.float32)


# ------------------------------------------------------------------- plan ---
class Plan:
    pass


def _make_plan(kernels):
    """Per-filter coverage plan + pairing schedule (same for every core)."""
    per_f = []
    for f in range(NF):
        nz = np.nonzero(kernels[f])[0]
        m = int(max(PADK - nz[0], nz[-1] - PADK)) if len(nz) else 0
        best = None
        for fam, OFF in ((0, 0), (1, -64)):
            jmin = math.floor((-m - OFF) / 128)
            jmax = math.floor((255 + m - OFF) / 128)
            nj = jmax - jmin + 1
            if best is None or nj < best[4]:
                best = (fam, OFF, jmin, jmax, nj)
        per_f.append(best)

    # group filters by signature (fam, jmin, J); pair within each class
    from collections import defaultdict
    classes = defaultdict(list)
    for f in range(NF):
        fam, OFF, jmin, jmax, nj = per_f[f]
        classes[(fam, jmin, nj)].append(f)

    schedule = []   # (filters_tuple, fam, jmin, J, width)
    for (fam, jmin, nj), fs in sorted(classes.items(), key=lambda kv: kv[0][2]):
        i = 0
        while i + 1 < len(fs):
            schedule.append(((fs[i], fs[i + 1]), fam, jmin, nj, 2 * N))
            i += 2
        if i < len(fs):
            schedule.append(((fs[i],), fam, jmin, nj, N))
    # solos (small W, long J) first: their W lands fast, and the pair tiles
    # stream in behind the solo compute.
    schedule.sort(key=lambda s: (len(s[0]), -s[3]))

    p = Plan()
    p.per_f = per_f
    p.schedule = schedule
    # W layout: schedule-order, per step a [128, width] block
    p.w_steps = []   # per schedule idx: list of col offsets per step
    col = 0
    for (fls, fam, jmin, J, width) in schedule:
        cols = []
        for t in range(J):
            cols.append(col)
            col += width
        p.w_steps.append(cols)
    p.w_cols = col

    # x layout ranges per family
    p.fam_off = [0, -64]
    p.fam_cmin = []
    p.fam_C = []
    for fam in range(2):
        ents = [s for s in schedule if s[1] == fam]
        jmin = min(s[2] for s in ents)
        jmax = max(s[2] + s[3] - 1 for s in ents)
        p.fam_cmin.append(jmin)
        p.fam_C.append(254 + jmax - jmin + 1)
    p.xb_cols = p.fam_C[0] + p.fam_C[1]
    return p


# -------------------------------------------------------------- host prep ---
def _prep_x(x, plan, npdt):
    """Block-column-major x, per-batch [famA | famB]: [128, B*xb_cols]."""
    xf = np.ascontiguousarray(x.reshape(B, T), dtype=np.float32)
    LPAD = 1024
    xp = np.zeros((B, LPAD + T + LPAD), np.float32)
    xp[:, LPAD:LPAD + T] = xf
    fams = []
    for fam in range(2):
        C = plan.fam_C[fam]
        start = LPAD + 128 * plan.fam_cmin[fam] + plan.fam_off[fam]
        v = np.lib.stride_tricks.as_strided(
            xp[:, start:], shape=(B, C, 128),
            strides=(xp.strides[0], 512, 4))
        fams.append(v)
    out = np.concatenate(fams, axis=1)            # [B, xb_cols, 128]
    out = np.ascontiguousarray(out.transpose(2, 0, 1), dtype=npdt)
    return out.reshape(128, B * plan.xb_cols)


def _build_W(h, OFF, j):
    r = np.arange(128)[:, None]
    s = np.arange(N)[None, :]
    k = PADK + 128 * j + OFF + r - s
    valid = (k >= 0) & (k < KLEN)
    W = np.zeros((128, N), np.float32)
    W[valid] = h[np.clip(k, 0, KLEN - 1)][valid]
    return W


def _prep_w(kernels, plan, npdt):
    """Moving-operand blocks, identical for all cores: [128, w_cols]."""
    Wc = np.zeros((128, plan.w_cols), np.float32)
    for gi, (fls, fam, jmin, J, width) in enumerate(plan.schedule):
        OFF = plan.fam_off[fam]
        for t in range(J):
            j = jmin + t
            col = plan.w_steps[gi][t]
            for si, f in enumerate(fls):
                if plan.per_f[f][2] <= j <= plan.per_f[f][3]:
                    Wc[:, col + si * N:col + (si + 1) * N] = \
                        _build_W(kernels[f], OFF, j)
    return np.ascontiguousarray(Wc.astype(npdt))


# ---------------------------------------------------------------- program ---
def _build_program(plan, mmdt):
    import concourse.bacc as bacc
    import concourse.mybir as mybir
    from concourse.tile import TileContext

    F32 = mybir.dt.float32

    nc = bacc.Bacc("TRN2", target_bir_lowering=False)
    outdt = F32 if os.environ.get("KERNEL_OUT", "f16") == "f32" else mybir.dt.float16
    x_d = nc.dram_tensor("x", [128, GB * plan.xb_cols], mmdt,
                         kind="ExternalInput")
    w_d = nc.dram_tensor("w", [128, plan.w_cols], mmdt, kind="ExternalInput")
    y_d = nc.dram_tensor("y", [GB, NF, T], outdt, kind="ExternalOutput")
    y_ap = y_d.ap()

    rr = [0]

    with TileContext(nc) as tc:
        engs3 = (nc.sync, nc.scalar, nc.gpsimd)
        with (
            tc.tile_pool(name="wconst", bufs=1) as wpool,
            tc.tile_pool(name="xconst", bufs=1) as xpool,
            tc.tile_pool(name="psum", bufs=8, space="PSUM") as ppool,
            tc.tile_pool(name="ev", bufs=10) as epool,
        ):
            # One W tile per schedule group (separate tiles -> matmuls only
            # wait for their own group's DMA), round-robin on the 3 queues.
            # W streams exclusively on sync+gpsimd (clean FIFO, no
            # head-of-line blocking from the small output descriptors);
            # early outputs go to scalar only, late ones round-robin.
            x_s = xpool.tile([128, GB * plan.xb_cols], mmdt)
            nc.sync.dma_start(x_s[:], x_d[:])
            w_tiles = []
            for gi, (fls, fam, jmin, J, width) in enumerate(plan.schedule):
                lo = plan.w_steps[gi][0]
                hi = plan.w_steps[gi][-1] + width
                wt = wpool.tile([128, hi - lo], mmdt, tag=f"w{gi}")
                engs3[gi % 3].dma_start(wt[:], w_d[:, lo:hi])
                w_tiles.append(wt)
            ngrp = len(plan.schedule)

            # groups outer, batches inner: each W tile feeds 4 batches of
            # matmuls as soon as it lands, so W streaming stays ahead of PE.
            for gi, (fls, fam, jmin, J, width) in enumerate(plan.schedule):
                cmin = plan.fam_cmin[fam]
                pss = [ppool.tile([128, width], F32, tag="ps",
                                  name=f"ps_{gi}_{bb}") for bb in range(GB)]
                for t in range(J):
                    j = jmin + t
                    rhs = w_tiles[gi][:, t * width:(t + 1) * width]
                    for b in range(GB):
                        fbase = (b * plan.xb_cols
                                 + (plan.fam_C[0] if fam == 1 else 0))
                        col0 = fbase + (j - cmin)
                        lhsT = x_s[:, col0:col0 + 255:2]
                        nc.tensor.matmul(pss[b][:], lhsT, rhs,
                                         start=(t == 0), stop=(t == J - 1))
                for b in range(GB):
                    ev = epool.tile([128, width], outdt, tag="ev")
                    if (gi * GB + b) % 5 != 4:
                        nc.vector.tensor_copy(ev[:], pss[b][:])
                    else:
                        nc.scalar.copy(ev[:], pss[b][:])
                    for si, f in enumerate(fls):
                        yv = y_ap[b, f].rearrange("(a n) -> a n", n=N)
                        eng = engs3[rr[0] % 3]
                        rr[0] += 1
                        eng.dma_start(yv, ev[:, si * N:(si + 1) * N])
    nc.finalize()
    return nc


# ----------------------------------------------------------------- kernel ---
def kernel(x, kernels=None):
    global LAST_RESULT
    from concourse.bass_utils import run_bass_kernel_spmd

    x = np.asarray(x, dtype=np.float32)
    if kernels is None:
        kernels = _build_kernels()
    kernels = np.asarray(kernels, dtype=np.float32)
    assert x.shape == (B, 1, T) and kernels.shape == (NF, KLEN)

    kind, mmdt, npdt = _dtype_cfg()
    ckey = (kind, os.environ.get("KERNEL_OUT", "f16"))
    if ckey not in _COMPILED:
        plan = _make_plan(kernels)
        nc = _build_program(plan, mmdt)
        _COMPILED[ckey] = (nc, plan)
    nc, plan = _COMPILED[ckey]

    xh = _prep_x(x, plan, npdt)
    w = _prep_w(kernels, plan, npdt)
    gcols = GB * plan.xb_cols
    in_maps = [{"x": np.ascontiguousarray(xh[:, c * gcols:(c + 1) * gcols]),
                "w": w} for c in range(NCORE)]

    trace = bool(int(os.environ.get("KERNEL_TRACE", "0")))
    try:
        res = run_bass_kernel_spmd(nc, in_maps, core_ids=list(range(NCORE)),
                                   trace=trace)
    except Exception:
        if not trace:
            raise
        res = run_bass_kernel_spmd(nc, in_maps, core_ids=list(range(NCORE)),
                                   trace=False)
    LAST_RESULT = res

    out = np.empty((B, 1, NF, T), np.float32)
    for c in range(NCORE):
        out[c * GB:(c + 1) * GB, 0] = res.results[c]["y"].astype(np.float32)
    return out


# revision 29
# speedup vs baseline: 1.3620x; 1.0008x over previous
"""Trainium2 Bass kernel for nn_CombinedBandPassFilter.

Computes y[b, 0, f, t] = sum_k x[b, 0, t+k-384] * kernels[f, k]  (conv1d,
'same' padding, K=769, 40 filters, B=32, T=32768).

Strategy (8 NeuronCores, batch-sharded: 4 batches x all 40 filters per core):
  Block-Toeplitz matmul formulation. Output chunked t = 256*a + s with
  a in [0,128) as the PSUM partition dim and s in [0,256) as the free dim:

      y[256a + s] = sum_j sum_r x[256a + 128j + OFF + r] * W_j[r, s]
      W_j[r, s]   = h[384 + 128j + OFF + r - s]

  The stationary matmul operand is a [128, 128] stride-2-column slice of x
  stored block-column-major in SBUF (Xmat[r, c] = x[128c + SHIFT + r]); the
  moving operand is the precomputed filter-Toeplitz block W_j. PSUM
  accumulates over j. Per-filter offset OFF in {0, -64} (two x layouts)
  minimizes the j-block count to ceil((2m + 256)/128) for true tap
  half-width m, exploiting the wildly varying filter supports (19..769).

  Filters with identical (OFF, jmin, J) signatures are paired side by side
  into N=512 matmul streams sharing one stationary load: the 30 short
  filters form 15 pairs, the 6 mid filters 3 pairs, l=4/l=5 one pair; only
  the two longest filters run solo at N=256.  Every core executes the
  identical schedule on its own 4 batches, so the SPMD program has zero
  per-core padding.  DMA traffic is spread across the SP/ACT HWDGE queues
  and the gpsimd SWDGE queue; PSUM evacuation is split between the vector
  and scalar engines.
"""

import math
import os
import numpy as np

B = 32
T = 32768
KLEN = 769
PADK = 384
N = 256          # output chunk size = psum free dim
NCORE = 8
NF = 40
GB = B // NCORE  # batches per core

_COMPILED = {}
LAST_RESULT = None   # BassKernelResults of the most recent run (for test.py)


def _dtype_cfg():
    import concourse.mybir as mybir
    kind = os.environ.get("KERNEL_DTYPE", "f16")
    if kind == "f32r":
        return kind, mybir.dt.float32r, np.float32
    if kind == "f16":
        return kind, mybir.dt.float16, np.float16
    if kind == "bf16":
        import ml_dtypes
        return kind, mybir.dt.bfloat16, ml_dtypes.bfloat16
    raise ValueError(kind)


# ---------------------------------------------------------------- filters ---
def _design_filter(fs, low_hz, high_hz, cycle):
    n_taps = int(cycle * fs / low_hz)
    if n_taps % 2 == 0:
        n_taps += 1
    m = (n_taps - 1) / 2.0
    k = np.arange(n_taps) - m
    fl, fh = low_hz / fs, high_hz / fs
    h = 2.0 * fh * np.sinc(2.0 * fh * k) - 2.0 * fl * np.sinc(2.0 * fl * k)
    w = 0.54 - 0.46 * np.cos(2.0 * np.pi * np.arange(n_taps) / (n_taps - 1))
    h = h * w
    fc = 0.5 * (low_hz + high_hz) / fs
    resp = np.abs(np.sum(h * np.exp(-2j * np.pi * fc * k)))
    return (h / resp).astype(np.float32)


def _build_kernels():
    FS, CYCLE_PHA, CYCLE_AMP = 512, 3, 6
    pha = [(l, l + 2) for l in range(2, 22)]
    amp = [(l, l + 20) for l in range(60, 160, 5)]
    filters = [_design_filter(FS, l, h, CYCLE_PHA) for (l, h) in pha]
    filters += [_design_filter(FS, l, h, CYCLE_AMP) for (l, h) in amp]
    max_len = max(f.shape[0] for f in filters)
    padded = []
    for f in filters:
        pad = max_len - f.shape[0]
        padded.append(np.pad(f, (pad // 2, pad - pad // 2)))
    return np.stack(padded).astype(np.float32)


# ------------------------------------------------------------------- plan ---
class Plan:
    pass


def _make_plan(kernels):
    """Per-filter coverage plan + pairing schedule (same for every core)."""
    per_f = []
    for f in range(NF):
        nz = np.nonzero(kernels[f])[0]
        m = int(max(PADK - nz[0], nz[-1] - PADK)) if len(nz) else 0
        best = None
        for fam, OFF in ((0, 0), (1, -64)):
            jmin = math.floor((-m - OFF) / 128)
            jmax = math.floor((255 + m - OFF) / 128)
            nj = jmax - jmin + 1
            if best is None or nj < best[4]:
                best = (fam, OFF, jmin, jmax, nj)
        per_f.append(best)

    # group filters by signature (fam, jmin, J); pair within each class
    from collections import defaultdict
    classes = defaultdict(list)
    for f in range(NF):
        fam, OFF, jmin, jmax, nj = per_f[f]
        classes[(fam, jmin, nj)].append(f)

    schedule = []   # (filters_tuple, fam, jmin, J, width)
    for (fam, jmin, nj), fs in sorted(classes.items(), key=lambda kv: kv[0][2]):
        i = 0
        while i + 1 < len(fs):
            schedule.append(((fs[i], fs[i + 1]), fam, jmin, nj, 2 * N))
            i += 2
        if i < len(fs):
            schedule.append(((fs[i],), fam, jmin, nj, N))
    # solos (small W, long J) first: their W lands fast, and the pair tiles
    # stream in behind the solo compute.
    schedule.sort(key=lambda s: (len(s[0]), -s[3]))

    p = Plan()
    p.per_f = per_f
    p.schedule = schedule
    # W layout: schedule-order, per step a [128, width] block
    p.w_steps = []   # per schedule idx: list of col offsets per step
    col = 0
    for (fls, fam, jmin, J, width) in schedule:
        cols = []
        for t in range(J):
            cols.append(col)
            col += width
        p.w_steps.append(cols)
    p.w_cols = col

    # x layout ranges per family
    p.fam_off = [0, -64]
    p.fam_cmin = []
    p.fam_C = []
    for fam in range(2):
        ents = [s for s in schedule if s[1] == fam]
        jmin = min(s[2] for s in ents)
        jmax = max(s[2] + s[3] - 1 for s in ents)
        p.fam_cmin.append(jmin)
        p.fam_C.append(254 + jmax - jmin + 1)
    p.xb_cols = p.fam_C[0] + p.fam_C[1]
    return p


# -------------------------------------------------------------- host prep ---
def _prep_x(x, plan, npdt):
    """Block-column-major x, per-batch [famA | famB]: [128, B*xb_cols]."""
    xf = np.ascontiguousarray(x.reshape(B, T), dtype=np.float32)
    LPAD = 1024
    xp = np.zeros((B, LPAD + T + LPAD), np.float32)
    xp[:, LPAD:LPAD + T] = xf
    fams = []
    for fam in range(2):
        C = plan.fam_C[fam]
        start = LPAD + 128 * plan.fam_cmin[fam] + plan.fam_off[fam]
        v = np.lib.stride_tricks.as_strided(
            xp[:, start:], shape=(B, C, 128),
            strides=(xp.strides[0], 512, 4))
        fams.append(v)
    out = np.concatenate(fams, axis=1)            # [B, xb_cols, 128]
    out = np.ascontiguousarray(out.transpose(2, 0, 1), dtype=npdt)
    return out.reshape(128, B * plan.xb_cols)


def _build_W(h, OFF, j):
    r = np.arange(128)[:, None]
    s = np.arange(N)[None, :]
    k = PADK + 128 * j + OFF + r - s
    valid = (k >= 0) & (k < KLEN)
    W = np.zeros((128, N), np.float32)
    W[valid] = h[np.clip(k, 0, KLEN - 1)][valid]
    return W


def _prep_w(kernels, plan, npdt):
    """Moving-operand blocks, identical for all cores: [128, w_cols]."""
    Wc = np.zeros((128, plan.w_cols), np.float32)
    for gi, (fls, fam, jmin, J, width) in enumerate(plan.schedule):
        OFF = plan.fam_off[fam]
        for t in range(J):
            j = jmin + t
            col = plan.w_steps[gi][t]
            for si, f in enumerate(fls):
                if plan.per_f[f][2] <= j <= plan.per_f[f][3]:
                    Wc[:, col + si * N:col + (si + 1) * N] = \
                        _build_W(kernels[f], OFF, j)
    return np.ascontiguousarray(Wc.astype(npdt))


# ---------------------------------------------------------------- program ---
def _build_program(plan, mmdt):
    import concourse.bacc as bacc
    import concourse.mybir as mybir
    from concourse.tile import TileContext

    F32 = mybir.dt.float32

    nc = bacc.Bacc("TRN2", target_bir_lowering=False)
    outdt = F32 if os.environ.get("KERNEL_OUT", "f16") == "f32" else mybir.dt.float16
    x_d = nc.dram_tensor("x", [128, GB * plan.xb_cols], mmdt,
                         kind="ExternalInput")
    w_d = nc.dram_tensor("w", [128, plan.w_cols], mmdt, kind="ExternalInput")
    y_d = nc.dram_tensor("y", [GB, NF, T], outdt, kind="ExternalOutput")
    y_ap = y_d.ap()

    rr = [0]

    with TileContext(nc) as tc:
        engs3 = (nc.sync, nc.scalar, nc.gpsimd)
        with (
            tc.tile_pool(name="wconst", bufs=1) as wpool,
            tc.tile_pool(name="xconst", bufs=1) as xpool,
            tc.tile_pool(name="psum", bufs=8, space="PSUM") as ppool,
            tc.tile_pool(name="ev", bufs=10) as epool,
        ):
            # One W tile per schedule group (separate tiles -> matmuls only
            # wait for their own group's DMA), round-robin on the 3 queues.
            # W streams exclusively on sync+gpsimd (clean FIFO, no
            # head-of-line blocking from the small output descriptors);
            # early outputs go to scalar only, late ones round-robin.
            x_s = xpool.tile([128, GB * plan.xb_cols], mmdt)
            nc.sync.dma_start(x_s[:], x_d[:])
            w_tiles = []
            for gi, (fls, fam, jmin, J, width) in enumerate(plan.schedule):
                lo = plan.w_steps[gi][0]
                hi = plan.w_steps[gi][-1] + width
                wt = wpool.tile([128, hi - lo], mmdt, tag=f"w{gi}")
                engs3[(gi + 1) % 3].dma_start(wt[:], w_d[:, lo:hi])
                w_tiles.append(wt)
            ngrp = len(plan.schedule)

            # groups outer, batches inner: each W tile feeds 4 batches of
            # matmuls as soon as it lands, so W streaming stays ahead of PE.
            for gi, (fls, fam, jmin, J, width) in enumerate(plan.schedule):
                cmin = plan.fam_cmin[fam]
                pss = [ppool.tile([128, width], F32, tag="ps",
                                  name=f"ps_{gi}_{bb}") for bb in range(GB)]
                for t in range(J):
                    j = jmin + t
                    rhs = w_tiles[gi][:, t * width:(t + 1) * width]
                    for b in range(GB):
                        fbase = (b * plan.xb_cols
                                 + (plan.fam_C[0] if fam == 1 else 0))
                        col0 = fbase + (j - cmin)
                        lhsT = x_s[:, col0:col0 + 255:2]
                        nc.tensor.matmul(pss[b][:], lhsT, rhs,
                                         start=(t == 0), stop=(t == J - 1))
                for b in range(GB):
                    ev = epool.tile([128, width], outdt, tag="ev")
                    if (gi * GB + b) % 5 != 4:
                        nc.vector.tensor_copy(ev[:], pss[b][:])
                    else:
                        nc.scalar.copy(ev[:], pss[b][:])
                    for si, f in enumerate(fls):
                        yv = y_ap[b, f].rearrange("(a n) -> a n", n=N)
                        eng = engs3[rr[0] % 3]
                        rr[0] += 1
                        eng.dma_start(yv, ev[:, si * N:(si + 1) * N])
    nc.finalize()
    return nc


# ----------------------------------------------------------------- kernel ---
def kernel(x, kernels=None):
    global LAST_RESULT
    from concourse.bass_utils import run_bass_kernel_spmd

    x = np.asarray(x, dtype=np.float32)
    if kernels is None:
        kernels = _build_kernels()
    kernels = np.asarray(kernels, dtype=np.float32)
    assert x.shape == (B, 1, T) and kernels.shape == (NF, KLEN)

    kind, mmdt, npdt = _dtype_cfg()
    ckey = (kind, os.environ.get("KERNEL_OUT", "f16"))
    if ckey not in _COMPILED:
        plan = _make_plan(kernels)
        nc = _build_program(plan, mmdt)
        _COMPILED[ckey] = (nc, plan)
    nc, plan = _COMPILED[ckey]

    xh = _prep_x(x, plan, npdt)
    w = _prep_w(kernels, plan, npdt)
    gcols = GB * plan.xb_cols
    in_maps = [{"x": np.ascontiguousarray(xh[:, c * gcols:(c + 1) * gcols]),
                "w": w} for c in range(NCORE)]

    trace = bool(int(os.environ.get("KERNEL_TRACE", "0")))
    try:
        res = run_bass_kernel_spmd(nc, in_maps, core_ids=list(range(NCORE)),
                                   trace=trace)
    except Exception:
        if not trace:
            raise
        res = run_bass_kernel_spmd(nc, in_maps, core_ids=list(range(NCORE)),
                                   trace=False)
    LAST_RESULT = res

    out = np.empty((B, 1, NF, T), np.float32)
    for c in range(NCORE):
        out[c * GB:(c + 1) * GB, 0] = res.results[c]["y"].astype(np.float32)
    return out
